# revision 9
# baseline (speedup 1.0000x reference)
"""Trainium2 Bass kernel for nn_FCPairedLayer — exact-triangle stream version.

Math (B=2, N=1024, C=128, H1=128, H2=64):
    a = x @ W1[:C] + b1     # [B,N,H1]  left-token contribution
    r = x @ W1[C:]          # [B,N,H1]  right-token contribution
    h1 = relu(a_i + r_j)                # per ordered pair (i,j), j > i
    h2 = relu(h1 @ W2 + b2)
    y[b,i,j] = h2 @ W3 + b3  for j > i, else 0.

Strategy (8 cores, SPMD — one program, per-core data):
  Exact triangle coverage with one fused tensor_scalar H-instruction per
  row.  Core c (gamma = c//2, beta = c%2) owns rows {4t + gamma} of batch
  beta.  Slot t (t = 0..255) computes row i = 4t + gamma against window
  tokens [4t, 1024) — static offsets/widths w_t = 1024 - 4t identical on
  every core; per-core identity comes only from HOST packing: xw tokens
  are rolled left by gamma so the program's static window slices line up
  with each core's rows.  <=4 junk cols per row (diagonal + wrap) are
  simply never read back by the host.  Total stream C = 131,584 pair-cols
  per core (vs 196,608 redundant cols in the old block scheme).

  H pieces are written at running offsets into a ring of [128, 8192] bf16
  SBUF tiles; downstream consumes uniform [128, 1024] chunks that ignore
  row boundaries entirely (W2/h2s/W3 are all column-independent).  W2
  matmuls write 2-bank [64, 1024] PSUM regions, two stream-groups stacked
  per [128, 1024] ph tile.  y is packed DENSE in PSUM: per ph-pair m the
  W3 matmul slides a window over one [128, 128] "w3band" tensor (W3 at
  cols 62/63) so W3 lands at lhsT columns (2v, 2v+1), v = m%32,
  accumulating into one [64, 1024] PSUM tile (start at v=0, stop at v=31)
  so 64 groups of y land as contiguous rows -> two [64, 1024] copies +
  DMAs cover the whole output.  b3 and the triangle mask are host-side.

  Engine balance knobs (env): slots t >= FC_TPOOL run their H-construct
  on GpSimd (Pool) instead of DVE; FC_H2S_DVE / FC_H2S_POOL move every
  k-th h2s PSUM->SBUF relu from ACT to DVE / Pool.

  Environment workaround: this walrus build accepts at most ONE sync-wait
  per instruction, so compile goes through a BIR rewrite that moves extra
  Tile-generated waits onto single-wait EventSemaphore carriers.
"""

import numpy as np
import ml_dtypes

B, N, C = 2, 1024, 128
H1, H2 = 128, 64
NCORES = 8
BF16 = ml_dtypes.bfloat16

NSLOT = 256          # H slots per core (one per owned row)
RING_W = 16384       # ring tile width (bf16 cols)
STREAM_C = sum(1024 - 4 * t for t in range(NSLOT))        # 131,584
NGROUP = STREAM_C // 1024                                 # 128 full groups
NPAIRM = NGROUP // 2                                      # 64 ph pairs
TAIL_W = STREAM_C - NGROUP * 1024                         # 512

LAST_PERF = {}


# Stream order: a short burst first (rows needing only the tail token
# chunks, ready before the full rT lands), then long/short pairs so DVE's
# H production rate stays uniform (~1092 cols per 2 instructions).
SLOT_ORDER = list(range(NSLOT - 1, 239, -1))
for _t in range(120):
    SLOT_ORDER += [_t, 239 - _t]


def _slot_layout():
    """Static (slot, ring-tile, offset, width) pieces in stream order."""
    pieces = []          # (slot, src_off, ring_idx, dst_off, width)
    s = 0
    for t in SLOT_ORDER:
        w = 1024 - 4 * t
        src = 4 * t
        while w > 0:
            r, o = divmod(s, RING_W)
            take = min(w, RING_W - o)
            pieces.append((t, src, r, o, take))
            s += take
            src += take
            w -= take
    return pieces


def _split_sync_waits(bir_json, limit=1):
    """Walrus in this toolchain rejects instructions carrying more than one
    sync-wait command; rewrite the BIR so each instruction keeps at most
    `limit` waits and the rest ride on preceding single-wait EventSemaphore
    instructions on the same engine."""
    import json

    data = json.loads(bir_json)
    for f in data.get("functions", []):
        for blk in f.get("blocks", []):
            out = []
            for ins in blk.get("instructions", []):
                si = ins.get("sync_info")
                ow = (si or {}).get("on_wait") or []
                if len(ow) > limit:
                    for k, w in enumerate(ow[:-limit]):
                        out.append({
                            "debug": ins.get("debug", 0),
                            "engine": ins["engine"],
                            "name": f"{ins['name']}-xw{k}",
                            "opcode": "EventSemaphore",
                            "sync_info": {"on_update": [], "on_wait": [w]},
                        })
                    si["on_wait"] = ow[-limit:]
                out.append(ins)
            blk["instructions"] = out
    return json.dumps(data).encode()


def _install_compile_patch():
    import concourse.bass_utils as bu
    import concourse.bass2jax as b2j

    if getattr(bu, "_fc_split_waits_patch", False):
        return
    orig = bu.compile_bir_kernel

    def patched(bir_json, tmpdir, neff_name="file.neff"):
        return orig(_split_sync_waits(bir_json), tmpdir, neff_name)

    bu._fc_split_waits_patch = True
    bu.compile_bir_kernel = patched
    b2j.compile_bir_kernel = patched


def _build_program():
    import os
    import concourse.bass as bass
    import concourse.mybir as mybir
    from concourse.tile import TileContext

    t_act = int(os.environ.get("FC_TACT", "242"))       # slots >= this on ACT
    h2s_dve = int(os.environ.get("FC_H2S_DVE", "0"))    # every k-th on DVE
    h2s_pool = int(os.environ.get("FC_H2S_POOL", "0"))  # every k-th on Pool
    h2s_dve_from = int(os.environ.get("FC_H2S_DVE_FROM", "60"))

    f32 = mybir.dt.float32
    bf16 = mybir.dt.bfloat16
    nc = bass.Bass()

    xr_d = nc.declare_dram_parameter("xr", [C, NSLOT], bf16, isOutput=False)
    xw_d = nc.declare_dram_parameter("xw", [C, 1024], bf16, isOutput=False)
    w1l_d = nc.declare_dram_parameter("w1l", [C, H1], bf16, isOutput=False)
    w1r_d = nc.declare_dram_parameter("w1r", [C, H1], bf16, isOutput=False)
    b1c_d = nc.declare_dram_parameter("b1c", [H1, 1], f32, isOutput=False)
    w2b_d = nc.declare_dram_parameter("w2b", [H1, H2], bf16, isOutput=False)
    b2s_d = nc.declare_dram_parameter("b2s", [128, 1], f32, isOutput=False)
    w3b_d = nc.declare_dram_parameter("w3b", [128, 64], bf16, isOutput=False)
    y_d = nc.declare_dram_parameter("y", [64, 2560], f32, isOutput=True)

    Relu = mybir.ActivationFunctionType.Relu
    ADD = mybir.AluOpType.add
    MAX = mybir.AluOpType.max

    pieces = _slot_layout()
    n_ring = pieces[-1][2] + 1           # 17

    with TileContext(nc) as tc:
        with tc.tile_pool(name="const", bufs=1) as const:
            w1l_t = const.tile([C, H1], bf16, tag="w1l")
            w1r_t = const.tile([C, H1], bf16, tag="w1r")
            b1c_t = const.tile([H1, 1], f32, tag="b1c")
            w2b_t = const.tile([H1, H2], bf16, tag="w2b")
            b2s_t = const.tile([128, 1], f32, tag="b2s")
            w3b_t = const.tile([128, 64], bf16, tag="w3b")
            xr_t = const.tile([C, NSLOT], bf16, tag="xr")
            xw_t = const.tile([C, 1024], bf16, tag="xw")
            aTb1_t = const.tile([H1, NSLOT], f32, tag="aTb1")
            rT_t = const.tile([H1, 1024], bf16, tag="rT")

            # DMA order follows the dependency chain of the opening short-slot
            # burst: the tiny b1c first (it gates aTb1 and hence all H), then
            # weights, the high token chunk (every row window ends at col
            # 1024), the burst slots' xr columns, then the rest.
            nc.sync.dma_start(out=b1c_t, in_=b1c_d[:])
            nc.sync.dma_start(out=w1r_t, in_=w1r_d[:])
            nc.sync.dma_start(out=xw_t[:, 768:1024], in_=xw_d[:, 768:1024])
            nc.sync.dma_start(out=xr_t[:, 192:256], in_=xr_d[:, 192:256])
            nc.sync.dma_start(out=w1l_t, in_=w1l_d[:])
            nc.sync.dma_start(out=xr_t[:, 0:192], in_=xr_d[:, 0:192])
            for ch in (2, 1, 0):
                nc.sync.dma_start(out=xw_t[:, ch * 256:(ch + 1) * 256],
                                  in_=xw_d[:, ch * 256:(ch + 1) * 256])
            for t, d in [(w2b_t, w2b_d), (b2s_t, b2s_d), (w3b_t, w3b_d)]:
                nc.sync.dma_start(out=t, in_=d[:])

            # Pre-stage: aT+b1 (f32 scalars for H bias) and rT (bf16 tokens).
            # aTb1 runs on ACT (idle at startup), burst columns first.
            Identity = mybir.ActivationFunctionType.Identity
            with tc.tile_pool(name="pre", bufs=2, space="PSUM") as pre:
                pr = pre.tile([128, 256], f32, tag="pr", name="pr3")
                nc.tensor.matmul(pr, lhsT=w1r_t, rhs=xw_t[:, 768:1024],
                                 start=True, stop=True)
                nc.scalar.copy(rT_t[:, 768:1024], pr)
                pa = pre.tile([128, NSLOT], f32, tag="pa")
                nc.tensor.matmul(pa[:, 192:256], lhsT=w1l_t,
                                 rhs=xr_t[:, 192:256], start=True, stop=True)
                nc.scalar.activation(aTb1_t[:, 192:256], pa[:, 192:256],
                                     Identity, bias=b1c_t)
                nc.tensor.matmul(pa[:, 0:192], lhsT=w1l_t,
                                 rhs=xr_t[:, 0:192], start=True, stop=True)
                nc.scalar.activation(aTb1_t[:, 0:192], pa[:, 0:192],
                                     Identity, bias=b1c_t)
                for ch in (2, 1, 0):
                    pr = pre.tile([128, 256], f32, tag="pr")
                    nc.tensor.matmul(pr, lhsT=w1r_t,
                                     rhs=xw_t[:, ch * 256:(ch + 1) * 256],
                                     start=True, stop=True)
                    nc.scalar.copy(rT_t[:, ch * 256:(ch + 1) * 256], pr)

            with (
                tc.tile_pool(name="Hp", bufs=2) as Hp,
                tc.tile_pool(name="h2p", bufs=6) as h2p,
                tc.tile_pool(name="ysp", bufs=2) as ysp,
                tc.tile_pool(name="php", bufs=2, space="PSUM") as php,
                tc.tile_pool(name="ypp", bufs=2, space="PSUM") as ypp,
            ):
                ring = [None] * n_ring
                piece_idx = 0
                yps = None
                h2s_n = 0

                def issue_pieces_for_tile(r):
                    nonlocal piece_idx
                    ring[r] = Hp.tile([128, RING_W], bf16, tag="H", name=f"Hr{r}")
                    while piece_idx < len(pieces) and pieces[piece_idx][2] == r:
                        t, src, _, o, w = pieces[piece_idx]
                        if t >= t_act:
                            nc.scalar.activation(ring[r][:, o:o + w],
                                                 rT_t[:, src:src + w], Relu,
                                                 bias=aTb1_t[:, t:t + 1])
                        else:
                            nc.vector.tensor_scalar(ring[r][:, o:o + w],
                                                    rT_t[:, src:src + w],
                                                    aTb1_t[:, t:t + 1], 0.0,
                                                    ADD, op1=MAX)
                        piece_idx += 1

                def h2s_engine(m):
                    nonlocal h2s_n
                    h2s_n += 1
                    # late-phase: DVE has finished H production and idles, so
                    # alternate the remaining h2s tiles onto it to speed the
                    # drain; ACT stays primary elsewhere.
                    if m >= h2s_dve_from and m % 2 == 1:
                        return "dve"
                    if h2s_dve and h2s_n % h2s_dve == 0:
                        return "dve"
                    if h2s_pool and h2s_n % h2s_pool == (h2s_pool // 2):
                        return "pool"
                    return "act"

                def issue_w3(m, h2s):
                    # deferred by one pair so h2s(m) is already done on ACT
                    nonlocal yps
                    v = m % 32
                    u, k = v % 16, v // 16
                    if v == 0:
                        yps = ypp.tile([64, 1024], f32, tag="yps",
                                       name=f"yps{m // 32}")
                    for q in range(2):
                        nc.tensor.matmul(
                            yps[32 * k:32 * (k + 1), 512 * q:512 * (q + 1)],
                            lhsT=w3b_t[:, 30 - 2 * u:62 - 2 * u],
                            rhs=h2s[:, 512 * q:512 * (q + 1)],
                            start=(u == 0), stop=(u == 15),
                            skip_group_check=True,
                            tile_position=(0, 32 * k))
                    if u == 15:
                        # quadrant k of fill f complete: copy + split DMAs.
                        # The final quadrant pipelines two half-copies so its
                        # DMAs start earlier (it is the exec-time tail).
                        f = m // 32
                        ysb = ysp.tile([32, 1024], f32, tag="ysb",
                                       name=f"ysb{2 * f + k}")
                        halves = 2 if m == NPAIRM - 1 else 1
                        hw = 1024 // halves
                        for c0 in range(halves):
                            nc.vector.tensor_copy(
                                ysb[:, hw * c0:hw * (c0 + 1)],
                                yps[32 * k:32 * (k + 1),
                                    hw * c0:hw * (c0 + 1)])
                            for h in range(4):
                                nc.sync.dma_start(
                                    out=y_d[32 * k + 8 * h:
                                            32 * k + 8 * (h + 1),
                                            1024 * f + hw * c0:
                                            1024 * f + hw * (c0 + 1)],
                                    in_=ysb[8 * h:8 * (h + 1),
                                            hw * c0:hw * (c0 + 1)])

                issue_pieces_for_tile(0)
                # tail group first: stream cols [0, 512) are produced by the
                # opening short-slot burst, so its whole chain retires early
                # and stays off the drain path.
                ph_t = php.tile([128, 1024], f32, tag="ph", name="ph_tail")
                nc.tensor.matmul(ph_t[0:64, 0:TAIL_W], lhsT=w2b_t,
                                 rhs=ring[0][:, 0:TAIL_W],
                                 start=True, stop=True)
                h2s_t = h2p.tile([64, 512], bf16, tag="h2st")
                nc.scalar.activation(h2s_t, ph_t[0:64, 0:TAIL_W], Relu,
                                     bias=b2s_t[0:64])
                yps_t = ypp.tile([64, 1024], f32, tag="yps", name="yps_t")
                nc.tensor.matmul(yps_t[0:1, 0:512], lhsT=w3b_t[0:64, 30:31],
                                 rhs=h2s_t, start=True, stop=True)
                ysb_t = ysp.tile([1, 512], f32, tag="ysbt")
                nc.vector.tensor_copy(ysb_t, yps_t[0:1, 0:512])
                nc.sync.dma_start(out=y_d[0:1, 2048:2560], in_=ysb_t)

                # full groups, 2-m blocks to amortize PE weight switches.
                # W3s run two blocks deferred and are issued BEFORE the W2s
                # of the current block: their h2s inputs are long done, so
                # they are guaranteed-ready filler work for the PE whenever
                # W2 stalls on the H-production frontier.
                pend, cur = [], []      # blocks of (m, h2s) awaiting W3
                for m in range(NPAIRM):
                    if m % 2 == 0 and len(pend) == 2:
                        for mm, hh in pend.pop(0):
                            issue_w3(mm, hh)
                    ph = php.tile([128, 1024], f32, tag="ph")
                    for e in range(2):
                        g = 2 * m + e
                        for q in range(2):
                            r, o = divmod(TAIL_W + 1024 * g + 512 * q,
                                          RING_W)
                            if ring[r] is None:
                                issue_pieces_for_tile(r)
                            nc.tensor.matmul(
                                ph[64 * e:64 * (e + 1),
                                   512 * q:512 * (q + 1)],
                                lhsT=w2b_t, rhs=ring[r][:, o:o + 512],
                                start=True, stop=True,
                                tile_position=(0, 64 * e))
                    h2s = h2p.tile([128, 1024], bf16, tag="h2s")
                    eng = h2s_engine(m)
                    if eng == "act":
                        nc.scalar.activation(h2s, ph, Relu, bias=b2s_t)
                    elif eng == "dve":
                        nc.vector.tensor_scalar(h2s, ph, b2s_t, 0.0, ADD,
                                                op1=MAX)
                    else:
                        nc.gpsimd.tensor_scalar(h2s, ph, b2s_t, 0.0, ADD,
                                                op1=MAX)
                    cur.append((m, h2s))
                    if m % 2 == 1:
                        pend.append(cur)
                        cur = []
                for blk in pend:
                    for mm, hh in blk:
                        issue_w3(mm, hh)
    return nc


def _pack_inputs(x, W1, b1, W2, b2, W3, b3):
    xT = np.ascontiguousarray(x.transpose(0, 2, 1)).astype(np.float32)  # [2,C,N]
    w1l = np.ascontiguousarray(W1[:C]).astype(BF16)
    w1r = np.ascontiguousarray(W1[C:]).astype(BF16)
    b1c = np.ascontiguousarray(b1.reshape(H1, 1)).astype(np.float32)
    w2b = np.ascontiguousarray(W2).astype(BF16)
    b2s = np.concatenate([b2, b2]).reshape(128, 1).astype(np.float32)
    w3band = np.zeros((128, 64), dtype=BF16)
    w3band[0:64, 30] = W3[:, 0].astype(BF16)
    w3band[64:128, 31] = W3[:, 0].astype(BF16)

    in_maps = []
    for c in range(NCORES):
        gamma, beta = c // 2, c % 2
        xw = np.roll(xT[beta], -gamma, axis=1).astype(BF16)
        rows = 4 * np.arange(NSLOT) + gamma
        xr = xT[beta][:, rows].astype(BF16)
        in_maps.append({
            "xr": np.ascontiguousarray(xr), "xw": np.ascontiguousarray(xw),
            "w1l": w1l, "w1r": w1r, "b1c": b1c, "w2b": w2b, "b2s": b2s,
            "w3b": w3band,
        })
    return in_maps


def _assemble(results, b3):
    y = np.zeros((B, N, N), dtype=np.float32)
    s_off = {}
    s = 0
    for t in SLOT_ORDER:
        s_off[t] = s
        s += 1024 - 4 * t
    b3v = np.float32(b3[0])
    for c in range(NCORES):
        gamma, beta = c // 2, c % 2
        out = results[c]["y"]                      # [64, 2560]
        stream = np.empty(STREAM_C, dtype=np.float32)
        stream[0:TAIL_W] = out[0, 2048:2560]
        stream[TAIL_W:TAIL_W + 65536] = out[:, 0:1024].reshape(-1)
        stream[TAIL_W + 65536:STREAM_C] = out[:, 1024:2048].reshape(-1)
        for t in range(NSLOT):
            i = 4 * t + gamma
            n = 1023 - i
            if n > 0:
                s = s_off[t]
                y[beta, i, i + 1:1024] = stream[s + 1:s + 1 + n] + b3v
    return y


def kernel(x, W1, b1, W2, b2, W3, b3):
    import os
    _install_compile_patch()
    from concourse.bass_utils import run_bass_kernel_spmd

    trace = bool(int(os.environ.get("FC_TRACE", "0")))
    nc = _build_program()
    in_maps = _pack_inputs(np.asarray(x), np.asarray(W1), np.asarray(b1),
                           np.asarray(W2), np.asarray(b2), np.asarray(W3),
                           np.asarray(b3))
    res = run_bass_kernel_spmd(nc, in_maps, core_ids=list(range(NCORES)),
                               trace=trace)
    LAST_PERF.clear()
    LAST_PERF.update({
        "exec_time_ns": res.exec_time_ns,
        "mean_exec_time_ns": res.mean_exec_time_ns,
        "trace": res.instructions_and_trace[1] if res.instructions_and_trace else None,
    })
    return _assemble(res.results, np.asarray(b3))
